# revision 25
# baseline (speedup 1.0000x reference)
"""GAT model Bass/Tile kernel for TRN2 (self-contained, octet-batched).

Per core: 512 graphs as 256 pairs (128 nodes / 112 edges). Pairs are
processed in octets (8 pairs): per-edge/per-node attention scalars are
batched into [*, 64] ops across the octet; fat value ops run at duet
(2-pair) granularity; engines are balanced DVE/ACT/GPSIMD/PE.
"""
import numpy as np
from contextlib import ExitStack

import concourse.bass as bass
import concourse.tile as tile
from concourse import bacc, mybir
from concourse.bass_utils import run_bass_kernel_spmd

F32 = mybir.dt.float32
I32 = mybir.dt.int32

B, A, OBS = 4096, 8, 56
P = 64
H, HID, HC = 8, 32, 256
IN, OUT = 16, 2
NCORES = 8
GPC = B // NCORES
EPP = 2 * OBS
ALU = mybir.AluOpType
ACTF = mybir.ActivationFunctionType

# small_ps column regions (f32); Z/ZG/oc reuse the same tile post-L3
ASD_, DEN_, RD_, CNT_, Z_, ZG_ = 0, 64, 128, 192, 0, 128


def build(npairs: int, vdt=mybir.dt.bfloat16, num_devices: int = NCORES):
    assert npairs % 8 == 0
    nc = bacc.Bacc("TRN2", target_bir_lowering=False, debug=False,
                   num_devices=num_devices)
    NP = npairs

    def din(name, shape, dt):
        return nc.dram_tensor(name, shape, dt, kind="ExternalInput").ap()

    xt = din("xt", [IN, NP * 128], vdt)
    esrcb = din("esrcb", [NP, EPP], vdt)
    edstb = din("edstb", [NP, EPP], vdt)
    edst = din("edst", [EPP, NP], F32)
    eattr = din("eattr", [EPP, NP], F32)
    eap = din("eap", [EPP, 2 * NP], vdt)
    waug1 = din("waug1", [IN, 272], vdt)
    waug2 = din("waug2", [128, 544], vdt)
    waug3 = din("waug3", [128, 544], vdt)
    webe = din("webe", [EPP, 3 * 64], F32)    # w_e tiled 8x per layer
    webn = din("webn", [128, 3 * 64], F32)
    fc1a = din("fc1a", [128, HC], vdt)
    fc1g = din("fc1g", [128, HC], vdt)
    fc1b = din("fc1b", [128, 1], F32)
    fc2w = din("fc2w", [128, OUT], vdt)
    fc2b = din("fc2b", [OUT, 1], F32)
    ident = din("ident", [128, 128], vdt)
    iota = din("iota", [EPP, 128], vdt)
    iotac = din("iotac", [128, 1], F32)

    out_d = nc.dram_tensor("out", [OUT, NP * 16], F32, kind="ExternalOutput").ap()

    with tile.TileContext(nc) as tc, ExitStack() as ctx:
        cpool = ctx.enter_context(tc.tile_pool(name="const", bufs=1))
        wk = ctx.enter_context(tc.tile_pool(name="work", bufs=4))
        eb = ctx.enter_context(tc.tile_pool(name="edges", bufs=24))
        ps = ctx.enter_context(tc.tile_pool(name="psum", bufs=1, space="PSUM"))

        def cload(ap, tag):
            t = cpool.tile(list(ap.shape), ap.dtype, tag=tag)
            nc.sync.dma_start(t[:], ap[:, :])
            return t

        c_w1, c_w2, c_w3 = cload(waug1, "w1"), cload(waug2, "w2"), cload(waug3, "w3")
        c_webe, c_webn = cload(webe, "webe"), cload(webn, "webn")
        c_fc1a, c_fc1g = cload(fc1a, "fc1a"), cload(fc1g, "fc1g")
        c_fc1b, c_fc2w, c_fc2b = cload(fc1b, "fc1b"), cload(fc2w, "fc2w"), cload(fc2b, "fc2b")
        c_id, c_iota = cload(ident, "ident"), cload(iota, "iota")
        c_iotac = cload(iotac, "iotac")
        c_edst = cload(edst, "edst")
        c_ea, c_eap = cload(eattr, "eattr"), cload(eap, "eap")

        out_acc = cpool.tile([OUT, NP * 16], F32, tag="out_acc")

        wchunks = {1: [c_w1[:, :]],
                   2: [c_w2[:, 0:272], c_w2[:, 272:544]],
                   3: [c_w3[:, 0:272], c_w3[:, 272:544]]}

        for oct_i in range(NP // 8):
            p0 = oct_i * 8

            # ---- phase A: edge structure + x loads ----
            srcb = eb.tile([128, 8 * EPP], vdt, tag="srcb", bufs=2)
            nc.sync.dma_start(srcb[:], esrcb[p0:p0 + 8, :]
                              .rearrange("a b -> (a b)")[None, :]
                              .broadcast_to([128, 8 * EPP]))
            dstb = eb.tile([128, 8 * EPP], vdt, tag="dstb", bufs=2)
            nc.sync.dma_start(dstb[:], edstb[p0:p0 + 8, :]
                              .rearrange("a b -> (a b)")[None, :]
                              .broadcast_to([128, 8 * EPP]))
            sblk_o = eb.tile([128, 8 * EPP], vdt, tag="sblk_o", bufs=2)
            nc.vector.tensor_scalar(sblk_o[:], srcb[:], c_iotac[:, 0:1],
                                    None, ALU.is_equal)
            dblk_o = eb.tile([128, 8 * EPP], vdt, tag="dblk_o", bufs=2)
            nc.vector.tensor_scalar(dblk_o[:], dstb[:], c_iotac[:, 0:1],
                                    None, ALU.is_equal)
            sblk_l = [sblk_o[:, j * EPP:(j + 1) * EPP] for j in range(8)]
            dblk_l = [dblk_o[:, j * EPP:(j + 1) * EPP] for j in range(8)]
            dtt_l, x0_l = [], []
            for j in range(8):
                pp = p0 + j
                dtt = eb.tile([EPP, 128], vdt, tag="dtt")
                nc.vector.tensor_scalar(dtt[:], c_iota[:], c_edst[:, pp:pp + 1],
                                        None, ALU.is_equal)
                x0 = eb.tile([IN, 128], vdt, tag="x0")
                nc.sync.dma_start(x0[:], xt[:, pp * 128:(pp + 1) * 128])
                dtt_l.append(dtt); x0_l.append(x0)

            prevT = [[x0_l[j][:, :]] for j in range(8)]
            ea_loop = None
            hT_l3 = None

            for li in (1, 2, 3):
                wch = wchunks[li]
                self_loops = li > 1
                sp = ps.tile([128, 208], F32, tag="small", bufs=2)
                hAso = wk.tile([128, 8 * 272], vdt, tag="hAso", bufs=4)
                hAv = hAso[:, :].rearrange("p (pr x) -> p pr x", pr=8)

                # ---- MM1 per pair + hAs copy + small gathers ----
                for j in range(8):
                    h2 = ps.tile([128, 272], F32, tag="h2", bufs=2)
                    for kc, (sta, wc) in enumerate(zip(prevT[j], wch)):
                        nc.tensor.matmul(h2[:], sta, wc, start=(kc == 0),
                                         stop=(kc == len(wch) - 1))
                    if li > 1 and j % 2 == 0:
                        nc.vector.tensor_copy(
                            hAso[:, j * 272:(j + 1) * 272], h2[:])
                    else:
                        nc.scalar.copy(
                            hAso[:, j * 272:(j + 1) * 272], h2[:])
                    nc.tensor.matmul(sp[0:112, ASD_ + j * 8:ASD_ + j * 8 + 8],
                                     sblk_l[j], hAv[:, j, 256:264],
                                     start=True, stop=False)
                    nc.tensor.matmul(sp[0:112, ASD_ + j * 8:ASD_ + j * 8 + 8],
                                     dblk_l[j], hAv[:, j, 264:272],
                                     start=False, stop=True)

                if li == 1:
                    for j in range(8):
                        nc.tensor.matmul(
                            sp[:, CNT_ + j * 2:CNT_ + j * 2 + 2], dtt_l[j][:],
                            c_eap[:, 2 * (p0 + j):2 * (p0 + j) + 2],
                            start=True, stop=True)
                    cntv = sp[:, CNT_:CNT_ + 16].rearrange(
                        "p (pr two) -> p pr two", two=2)
                    cntm = wk.tile([128, 8], F32, tag="cntm")
                    nc.vector.tensor_scalar(cntm[:], cntv[:, :, 1:2], 1.0,
                                            None, ALU.max)
                    rc = wk.tile([128, 8], F32, tag="rc")
                    nc.vector.reciprocal(rc[:], cntm[:])
                    ea_loop = wk.tile([128, 8], F32, tag="ea_loop")
                    nc.vector.tensor_tensor(ea_loop[:], cntv[:, :, 0:1]
                                            .rearrange("p a b -> p (a b)"),
                                            rc[:], ALU.mult)

                # ---- batched edge logits ----
                ae = wk.tile([EPP, 64], F32, tag="ae")
                nc.gpsimd.tensor_tensor(
                    ae[:].rearrange("p (a h) -> p a h", a=8),
                    c_webe[:, (li - 1) * 64:li * 64]
                    .rearrange("p (a h) -> p a h", a=8),
                    c_ea[:, p0:p0 + 8][:, :, None].broadcast_to([EPP, 8, H]),
                    ALU.mult)
                lg = wk.tile([EPP, 64], F32, tag="lg")
                nc.vector.scalar_tensor_tensor(
                    lg[:], sp[0:112, ASD_:ASD_ + 64], 1.0, ae[:],
                    ALU.mult, ALU.add)
                lg2 = wk.tile([EPP, 64], F32, tag="lg2")
                nc.vector.scalar_tensor_tensor(
                    lg2[:], lg[:], 0.2, lg[:], ALU.mult, ALU.max)
                p_e = wk.tile([EPP, 64], vdt, tag="p_e")
                nc.scalar.activation(p_e[:], lg2[:], ACTF.Exp)

                p_self = None
                if self_loops:
                    sae = wk.tile([128, 64], F32, tag="sae")
                    nc.gpsimd.tensor_tensor(
                        sae[:].rearrange("p (a h) -> p a h", a=8),
                        c_webn[:, (li - 1) * 64:li * 64]
                        .rearrange("p (a h) -> p a h", a=8),
                        ea_loop[:][:, :, None].broadcast_to([128, 8, H]),
                        ALU.mult)
                    s1 = wk.tile([128, 64], F32, tag="s1")
                    nc.gpsimd.tensor_tensor(
                        s1[:].rearrange("p (a h) -> p a h", a=8),
                        hAv[:, :, 256:264],
                        hAv[:, :, 264:272], ALU.add)
                    s2 = wk.tile([128, 64], F32, tag="s2")
                    nc.gpsimd.tensor_tensor(
                        s2[:], s1[:], sae[:], ALU.add)
                    s3 = wk.tile([128, 64], F32, tag="s3")
                    nc.vector.scalar_tensor_tensor(
                        s3[:], s2[:], 0.2, s2[:], ALU.mult, ALU.max)
                    p_self = wk.tile([128, 64], F32, tag="p_self")
                    nc.scalar.activation(p_self[:], s3[:], ACTF.Exp)

                # ---- denominators ----
                for j in range(8):
                    nc.tensor.matmul(sp[:, DEN_ + j * 8:DEN_ + j * 8 + 8],
                                     dtt_l[j][:], p_e[:, j * 8:j * 8 + 8],
                                     start=True, stop=True)
                dtot = wk.tile([128, 64], F32, tag="dtot")
                if self_loops:
                    nc.vector.tensor_tensor(dtot[:], sp[:, DEN_:DEN_ + 64],
                                            p_self[:], ALU.add)
                else:
                    nc.vector.tensor_scalar(dtot[:], sp[:, DEN_:DEN_ + 64],
                                            1e-16, None, ALU.add)
                rden = wk.tile([128, 64], F32, tag="rden")
                nc.vector.reciprocal(rden[:], dtot[:])
                rden_v = wk.tile([128, 64], vdt, tag="rden_v")
                nc.scalar.copy(rden_v[:], rden[:])
                for j in range(8):
                    nc.tensor.matmul(sp[0:112, RD_ + j * 8:RD_ + j * 8 + 8],
                                     dblk_l[j], rden_v[:, j * 8:j * 8 + 8],
                                     start=True, stop=True)
                pn = wk.tile([EPP, 64], vdt, tag="pn")
                nc.vector.tensor_tensor(pn[:], sp[0:112, RD_:RD_ + 64], p_e[:],
                                        ALU.mult)
                if self_loops:
                    psn = wk.tile([128, 64], vdt, tag="psn")
                    nc.gpsimd.tensor_tensor(psn[:], p_self[:], rden[:], ALU.mult)

                # ---- phase B per duet: gather h, messages, scatter, relu ----
                hT_new = []
                for d in range(4):
                    g2 = ps.tile([EPP, 512], F32, tag="g2", bufs=2)
                    for jj in range(2):
                        j = 2 * d + jj
                        nc.tensor.matmul(g2[:, jj * 256:jj * 256 + 256],
                                         sblk_l[j], hAv[:, j, 0:256],
                                         start=True, stop=True)
                    msg2 = wk.tile([EPP, 512], vdt, tag="msg2")
                    nc.vector.tensor_tensor(
                        msg2[:].rearrange("p (a h c) -> p a h c", a=2, h=H),
                        g2[:, :].rearrange("p (a h c) -> p a h c", a=2, h=H),
                        pn[:, d * 16:(d + 1) * 16]
                        .rearrange("p (a h) -> p a h", a=2)[:, :, :, None]
                        .broadcast_to([EPP, 2, H, HID]), ALU.mult)
                    if self_loops:
                        msgs2 = wk.tile([128, 512], vdt, tag="msgs2")
                        for jj in range(2):
                            j = 2 * d + jj
                            nc.gpsimd.tensor_tensor(
                                msgs2[:, jj * 256:(jj + 1) * 256]
                                .rearrange("p (h c) -> p h c", h=H),
                                hAv[:, j, 0:256]
                                .rearrange("p (h c) -> p h c", h=H),
                                psn[:, j * 8:(j + 1) * 8][:, :, None]
                                .broadcast_to([128, H, HID]), ALU.mult)
                    o2 = ps.tile([128, 512], F32, tag="out2", bufs=2)
                    for jj in range(2):
                        j = 2 * d + jj
                        for c in range(2):
                            cs = slice(jj * 256 + c * 128, jj * 256 + c * 128 + 128)
                            nc.tensor.matmul(
                                o2[:, cs], msg2[:, jj * 256 + c * 128:
                                                jj * 256 + (c + 1) * 128],
                                dtt_l[j][:], start=True, stop=not self_loops)
                            if self_loops:
                                nc.tensor.matmul(
                                    o2[:, cs], msgs2[:, jj * 256 + c * 128:
                                                     jj * 256 + (c + 1) * 128],
                                    c_id[:, :], start=False, stop=True)
                    hT2 = wk.tile([128, 512], vdt, tag=f"hT{li}_{d}")
                    nc.scalar.activation(hT2[:], o2[:], ACTF.Relu)
                    hT_new.append(hT2)
                    for jj in range(2):
                        j = 2 * d + jj
                        prevT[j] = [hT2[:, jj * 256:jj * 256 + 128],
                                    hT2[:, jj * 256 + 128:jj * 256 + 256]]
                hT_l3 = hT_new

            # ---- pooling + MLP (octet-batched) ----
            zmlp = ps.tile([128, 144], F32, tag="g2", bufs=2)
            gev_o = wk.tile([128, 32], F32, tag="gev_o")   # (c, pair, g)
            gvv = gev_o[:, :].rearrange("p (c pr g) -> p pr c g", c=2, g=2)
            for d in range(4):
                nc.vector.tensor_reduce(
                    gvv[:, 2 * d:2 * d + 2, :, :],
                    hT_l3[d][:, :].rearrange(
                        "p (a b g n) -> p a b g n", a=2, b=2, g=2),
                    mybir.AxisListType.X, ALU.add)
            gev_v = wk.tile([128, 32], vdt, tag="gev_v")
            nc.scalar.copy(gev_v[:], gev_o[:])
            for d in range(4):
                for c in range(2):
                    agent = hT_l3[d][:, :].rearrange(
                        "p (a b g n) -> p a b g n", a=2, b=2, g=2)[:, :, c, :, 0:8]
                    nc.tensor.matmul(zmlp[:, Z_ + d * 32:Z_ + (d + 1) * 32],
                                     c_fc1a[:, bass.ts(c, 128)], agent,
                                     start=(c == 0), stop=(c == 1))
            for c in range(2):
                nc.tensor.matmul(zmlp[:, ZG_:ZG_ + 16],
                                 c_fc1g[:, bass.ts(c, 128)],
                                 gev_v[:, c * 16:(c + 1) * 16],
                                 start=(c == 0), stop=(c == 1))
            zgb = wk.tile([128, 16], F32, tag="zgb")
            nc.vector.scalar_tensor_tensor(
                zgb[:], zmlp[:, ZG_:ZG_ + 16], 1.0,
                c_fc1b[:, 0:1].broadcast_to([128, 16]), ALU.mult, ALU.add)
            zt = wk.tile([128, 128], F32, tag="zt")
            nc.vector.scalar_tensor_tensor(
                zt[:].rearrange("p (a b) -> p a b", a=16),
                zmlp[:, Z_:Z_ + 128].rearrange("p (a b) -> p a b", a=16), 1.0,
                zgb[:][:, :, None].broadcast_to([128, 16, 8]),
                ALU.mult, ALU.add)
            zbat = wk.tile([128, 128], vdt, tag="zbat")
            nc.scalar.activation(zbat[:], zt[:], ACTF.Relu)
            nc.tensor.matmul(zmlp[0:OUT, 0:128], c_fc2w[:, :], zbat[:],
                             start=True, stop=True)
            nc.vector.tensor_scalar(out_acc[:, oct_i * 128:(oct_i + 1) * 128],
                                    zmlp[0:OUT, 0:128], c_fc2b[:, 0:1], None,
                                    ALU.add)

        nc.sync.dma_start(out_d[:, :], out_acc[:])

    nc.compile()
    return nc


# ---------------- host-side packing ----------------

def _np_vdt(vdt):
    import ml_dtypes
    return {mybir.dt.bfloat16: ml_dtypes.bfloat16,
            mybir.dt.float32: np.float32}[vdt]


def host_prep(inputs, npairs=GPC // 2, vdt=mybir.dt.bfloat16):
    nv = _np_vdt(vdt)
    x = np.asarray(inputs["x"], np.float32)
    ei = np.asarray(inputs["edge_index"])
    eattr = np.asarray(inputs["edge_attr"], np.float32)
    for l in (1, 2, 3):
        assert not np.any(np.asarray(inputs[f"b{l}"])), "GAT bias must be 0"

    def pack_w(l):
        W = np.asarray(inputs[f"W{l}"], np.float32)
        a_s = np.asarray(inputs[f"as{l}"], np.float32)
        a_d = np.asarray(inputs[f"ad{l}"], np.float32)
        Ps = np.einsum("fkc,kc->fk", W.reshape(W.shape[0], H, HID), a_s)
        Pd = np.einsum("fkc,kc->fk", W.reshape(W.shape[0], H, HID), a_d)
        return np.concatenate([W, Ps, Pd], axis=1).astype(nv)

    def w_e(l):
        We = np.asarray(inputs[f"We{l}"], np.float32).reshape(H, HID)
        a_e = np.asarray(inputs[f"ae{l}"], np.float32)
        return (We * a_e).sum(-1)

    waug = {l: pack_w(l) for l in (1, 2, 3)}
    for l in (2, 3):
        waug[l] = np.concatenate([waug[l][:128], waug[l][128:]], axis=1)
    wev = np.concatenate([np.tile(w_e(l), 8) for l in (1, 2, 3)])   # [192]
    webe = np.broadcast_to(wev, (EPP, 192)).astype(np.float32).copy()
    webn = np.broadcast_to(wev, (128, 192)).astype(np.float32).copy()
    fc1_w = np.asarray(inputs["fc1_w"], np.float32)
    fc1a = np.concatenate([fc1_w[:128], fc1_w[128:HC]], axis=1).astype(nv)
    fc1g = np.concatenate([fc1_w[HC:HC + 128] / P,
                           fc1_w[HC + 128:] / P], axis=1).astype(nv)
    fc1b = np.asarray(inputs["fc1_b"], np.float32).reshape(128, 1)
    fc2w = np.asarray(inputs["fc2_w"], np.float32).astype(nv)
    fc2b = np.asarray(inputs["fc2_b"], np.float32).reshape(OUT, 1)
    identm = np.eye(128, dtype=np.float32).astype(nv)
    iota = np.broadcast_to(np.arange(128, dtype=np.float32),
                           (EPP, 128)).astype(nv).copy()
    iotac = np.arange(128, dtype=np.float32).reshape(128, 1)

    maps = []
    npc = GPC * P
    epc = GPC * OBS
    for m in range(NCORES):
        nsl = slice(m * npc, (m + 1) * npc)
        esl = slice(m * epc, (m + 1) * epc)
        xt = np.ascontiguousarray(x[nsl].T).astype(nv)
        src = np.asarray(ei[0][esl], np.int64) - m * npc
        dst = np.asarray(ei[1][esl], np.int64) - m * npc
        pairs = np.arange(GPC // 2).repeat(EPP)
        src_l = (src.reshape(-1) - pairs * 128).astype(np.float32)
        dst_l = (dst.reshape(-1) - pairs * 128).astype(np.float32)
        esrcb = np.ascontiguousarray(src_l.reshape(-1, EPP)).astype(nv)
        edstb = np.ascontiguousarray(dst_l.reshape(-1, EPP)).astype(nv)
        edst = np.ascontiguousarray(dst_l.reshape(-1, EPP).T)
        eat = np.ascontiguousarray(eattr[esl].reshape(-1, EPP).T).astype(np.float32)
        eap_arr = np.empty((EPP, 2 * npairs), np.float32)
        eap_arr[:, 0::2] = eat[:, :npairs]
        eap_arr[:, 1::2] = 1.0
        maps.append({
            "xt": xt[:, :npairs * 128],
            "esrcb": esrcb[:npairs], "edstb": edstb[:npairs],
            "edst": edst[:, :npairs],
            "eattr": eat[:, :npairs], "eap": eap_arr.astype(nv),
            "waug1": waug[1], "waug2": waug[2], "waug3": waug[3],
            "webe": webe, "webn": webn,
            "fc1a": fc1a, "fc1g": fc1g, "fc1b": fc1b,
            "fc2w": fc2w, "fc2b": fc2b,
            "ident": identm, "iota": iota, "iotac": iotac,
        })
    return maps


def unpack_out(res_list, npairs=GPC // 2):
    outs = []
    for m in range(NCORES):
        o = res_list[m]["out"]
        o = o.reshape(OUT, npairs, 2, A).transpose(1, 2, 3, 0)
        outs.append(o.reshape(npairs * 2, A, OUT))
    return np.concatenate(outs, axis=0).astype(np.float32)


# ---------------- entry point ----------------

LAST_EXEC_NS = None
LAST_TRACE = None
_NC_CACHE = {}


def _install_trace_hook():
    """Best-effort: register the axon NTFF profile hook so trace=True works.

    The agent image's antenv lacks axon_hooks; fabricate it and wire the
    ctypes hook from trn_agent_boot. Silently a no-op anywhere else.
    """
    try:
        import sys
        import types
        if 'antenv.axon_hooks' not in sys.modules:
            import antenv
            mod = types.ModuleType('antenv.axon_hooks')
            _h = [None]
            mod.set_axon_ntff_profile_hook = lambda h: _h.__setitem__(0, h)
            mod.get_axon_ntff_profile_hook = lambda: _h[0]
            sys.modules['antenv.axon_hooks'] = mod
            antenv.axon_hooks = mod
        import antenv.axon_hooks as ah
        if ah.get_axon_ntff_profile_hook() is None:
            if '/root/.axon_site' not in sys.path:
                sys.path.insert(0, '/root/.axon_site')
            from trn_agent_boot.trn_boot import _ntff_profile_via_ctypes
            hook = _ntff_profile_via_ctypes('/opt/axon/libaxon_pjrt.so')
            if hook is not None:
                ah.set_axon_ntff_profile_hook(hook)
    except Exception:
        pass


def kernel(**inputs) -> np.ndarray:
    """Full-input GAT forward on 8 NeuronCores; returns [4096, 8, 2] f32."""
    global LAST_EXEC_NS, LAST_TRACE
    import os
    vdt = mybir.dt.bfloat16
    npairs = GPC // 2
    key = (npairs, vdt)
    if key not in _NC_CACHE:
        _NC_CACHE[key] = build(npairs, vdt=vdt, num_devices=NCORES)
    nc = _NC_CACHE[key]
    maps = host_prep(inputs, npairs=npairs, vdt=vdt)
    trace = os.environ.get("BASS_GAT_TRACE") == "1"
    if trace:
        _install_trace_hook()
    res = None
    for attempt in range(3):
        try:
            res = run_bass_kernel_spmd(
                nc, maps, core_ids=list(range(NCORES)),
                trace=trace and attempt == 0,
                trace_cores=[0] if trace and attempt == 0 else None)
            break
        except Exception:
            if attempt == 2:
                raise
            import time
            time.sleep(10)
    LAST_EXEC_NS = res.exec_time_ns
    LAST_TRACE = res.instructions_and_trace
    return unpack_out([r for r in res.results], npairs=npairs)


# revision 26
# speedup vs baseline: 1.0848x; 1.0848x over previous
"""GAT model Bass/Tile kernel for TRN2 (self-contained, octet-batched).

Per core: 512 graphs as 256 pairs (128 nodes / 112 edges). Pairs are
processed in octets (8 pairs): per-edge/per-node attention scalars are
batched into [*, 64] ops across the octet; fat value ops run at duet
(2-pair) granularity; engines are balanced DVE/ACT/GPSIMD/PE.
"""
import numpy as np
from contextlib import ExitStack

import concourse.bass as bass
import concourse.tile as tile
from concourse import bacc, mybir
from concourse.bass_utils import run_bass_kernel_spmd

F32 = mybir.dt.float32
I32 = mybir.dt.int32

B, A, OBS = 4096, 8, 56
P = 64
H, HID, HC = 8, 32, 256
IN, OUT = 16, 2
NCORES = 8
GPC = B // NCORES
EPP = 2 * OBS
ALU = mybir.AluOpType
ACTF = mybir.ActivationFunctionType

# small_ps column regions (f32); Z/ZG/oc reuse the same tile post-L3
ASD_, DEN_, RD_, CNT_, Z_, ZG_ = 0, 64, 128, 192, 0, 128


def build(npairs: int, vdt=mybir.dt.bfloat16, num_devices: int = NCORES):
    assert npairs % 8 == 0
    nc = bacc.Bacc("TRN2", target_bir_lowering=False, debug=False,
                   num_devices=num_devices)
    NP = npairs

    def din(name, shape, dt):
        return nc.dram_tensor(name, shape, dt, kind="ExternalInput").ap()

    xt = din("xt", [IN, NP * 128], vdt)
    esrcb = din("esrcb", [NP, EPP], vdt)
    edstb = din("edstb", [NP, EPP], vdt)
    edst = din("edst", [EPP, NP], F32)
    eattr = din("eattr", [EPP, NP], F32)
    eap = din("eap", [EPP, 2 * NP], vdt)
    waug1 = din("waug1", [IN, 272], vdt)
    waug2 = din("waug2", [128, 544], vdt)
    waug3 = din("waug3", [128, 544], vdt)
    webe = din("webe", [EPP, 3 * 64], F32)    # w_e tiled 8x per layer
    webn = din("webn", [128, 3 * 64], F32)
    fc1a = din("fc1a", [128, HC], vdt)
    fc1g = din("fc1g", [128, HC], vdt)
    fc1b = din("fc1b", [128, 1], F32)
    fc2w = din("fc2w", [128, OUT], vdt)
    fc2b = din("fc2b", [OUT, 1], F32)
    ident = din("ident", [128, 128], vdt)
    iota = din("iota", [EPP, 128], vdt)
    iotac = din("iotac", [128, 1], F32)

    out_d = nc.dram_tensor("out", [OUT, NP * 16], F32, kind="ExternalOutput").ap()

    with tile.TileContext(nc) as tc, ExitStack() as ctx:
        cpool = ctx.enter_context(tc.tile_pool(name="const", bufs=1))
        wk = ctx.enter_context(tc.tile_pool(name="work", bufs=4))
        eb = ctx.enter_context(tc.tile_pool(name="edges", bufs=24))
        ps = ctx.enter_context(tc.tile_pool(name="psum", bufs=1, space="PSUM"))

        def cload(ap, tag):
            t = cpool.tile(list(ap.shape), ap.dtype, tag=tag)
            nc.sync.dma_start(t[:], ap[:, :])
            return t

        c_w1, c_w2, c_w3 = cload(waug1, "w1"), cload(waug2, "w2"), cload(waug3, "w3")
        c_webe, c_webn = cload(webe, "webe"), cload(webn, "webn")
        c_fc1a, c_fc1g = cload(fc1a, "fc1a"), cload(fc1g, "fc1g")
        c_fc1b, c_fc2w, c_fc2b = cload(fc1b, "fc1b"), cload(fc2w, "fc2w"), cload(fc2b, "fc2b")
        c_id, c_iota = cload(ident, "ident"), cload(iota, "iota")
        c_iotac = cload(iotac, "iotac")
        c_edst = cload(edst, "edst")
        c_ea, c_eap = cload(eattr, "eattr"), cload(eap, "eap")

        out_acc = cpool.tile([OUT, NP * 16], F32, tag="out_acc")

        wchunks = {1: [c_w1[:, :]],
                   2: [c_w2[:, 0:272], c_w2[:, 272:544]],
                   3: [c_w3[:, 0:272], c_w3[:, 272:544]]}

        for oct_i in range(NP // 8):
            p0 = oct_i * 8

            # ---- phase A: edge structure + x loads ----
            srcb = eb.tile([128, 8 * EPP], vdt, tag="srcb", bufs=2)
            nc.sync.dma_start(srcb[:], esrcb[p0:p0 + 8, :]
                              .rearrange("a b -> (a b)")[None, :]
                              .broadcast_to([128, 8 * EPP]))
            dstb = eb.tile([128, 8 * EPP], vdt, tag="dstb", bufs=2)
            nc.sync.dma_start(dstb[:], edstb[p0:p0 + 8, :]
                              .rearrange("a b -> (a b)")[None, :]
                              .broadcast_to([128, 8 * EPP]))
            sblk_o = eb.tile([128, 8 * EPP], vdt, tag="sblk_o", bufs=2)
            nc.vector.tensor_scalar(sblk_o[:], srcb[:], c_iotac[:, 0:1],
                                    None, ALU.is_equal)
            dblk_o = eb.tile([128, 8 * EPP], vdt, tag="dblk_o", bufs=2)
            nc.vector.tensor_scalar(dblk_o[:], dstb[:], c_iotac[:, 0:1],
                                    None, ALU.is_equal)
            sblk_l = [sblk_o[:, j * EPP:(j + 1) * EPP] for j in range(8)]
            dblk_l = [dblk_o[:, j * EPP:(j + 1) * EPP] for j in range(8)]
            dtt_l, x0_l = [], []
            for j in range(8):
                pp = p0 + j
                dtt = eb.tile([EPP, 128], vdt, tag="dtt")
                nc.vector.tensor_scalar(dtt[:], c_iota[:], c_edst[:, pp:pp + 1],
                                        None, ALU.is_equal)
                x0 = eb.tile([IN, 128], vdt, tag="x0")
                nc.sync.dma_start(x0[:], xt[:, pp * 128:(pp + 1) * 128])
                dtt_l.append(dtt); x0_l.append(x0)

            prevT = [[x0_l[j][:, :]] for j in range(8)]
            ea_loop = None
            hT_l3 = None

            for li in (1, 2, 3):
                wch = wchunks[li]
                self_loops = li > 1
                sp = ps.tile([128, 208], F32, tag="small", bufs=2)
                hAso = wk.tile([128, 8 * 272], vdt, tag="hAso", bufs=4)
                hAv = hAso[:, :].rearrange("p (pr x) -> p pr x", pr=8)

                # ---- MM1 per pair + hAs copy + small gathers ----
                for j in range(8):
                    h2 = ps.tile([128, 272], F32, tag="h2", bufs=3)
                    for kc, (sta, wc) in enumerate(zip(prevT[j], wch)):
                        nc.tensor.matmul(h2[:], sta, wc, start=(kc == 0),
                                         stop=(kc == len(wch) - 1))
                    if j % 4 == 0:
                        nc.vector.tensor_copy(
                            hAso[:, j * 272:(j + 1) * 272], h2[:])
                    else:
                        nc.scalar.copy(
                            hAso[:, j * 272:(j + 1) * 272], h2[:])
                    nc.tensor.matmul(sp[0:112, ASD_ + j * 8:ASD_ + j * 8 + 8],
                                     sblk_l[j], hAv[:, j, 256:264],
                                     start=True, stop=False)
                    nc.tensor.matmul(sp[0:112, ASD_ + j * 8:ASD_ + j * 8 + 8],
                                     dblk_l[j], hAv[:, j, 264:272],
                                     start=False, stop=True)
                    if li == 1:
                        nc.tensor.matmul(
                            sp[:, CNT_ + j * 2:CNT_ + j * 2 + 2], dtt_l[j][:],
                            c_eap[:, 2 * (p0 + j):2 * (p0 + j) + 2],
                            start=True, stop=True)

                if li == 1:
                    cntv = sp[:, CNT_:CNT_ + 16].rearrange(
                        "p (pr two) -> p pr two", two=2)
                    cntm = wk.tile([128, 8], F32, tag="cntm")
                    nc.vector.tensor_scalar(cntm[:], cntv[:, :, 1:2], 1.0,
                                            None, ALU.max)
                    rc = wk.tile([128, 8], F32, tag="rc")
                    nc.vector.reciprocal(rc[:], cntm[:])
                    ea_loop = wk.tile([128, 8], F32, tag="ea_loop")
                    nc.vector.tensor_tensor(ea_loop[:], cntv[:, :, 0:1]
                                            .rearrange("p a b -> p (a b)"),
                                            rc[:], ALU.mult)

                # ---- batched edge logits ----
                ae = wk.tile([EPP, 64], F32, tag="ae")
                nc.gpsimd.tensor_tensor(
                    ae[:].rearrange("p (a h) -> p a h", a=8),
                    c_webe[:, (li - 1) * 64:li * 64]
                    .rearrange("p (a h) -> p a h", a=8),
                    c_ea[:, p0:p0 + 8][:, :, None].broadcast_to([EPP, 8, H]),
                    ALU.mult)
                lg = wk.tile([EPP, 64], F32, tag="lg")
                nc.vector.scalar_tensor_tensor(
                    lg[:], sp[0:112, ASD_:ASD_ + 64], 1.0, ae[:],
                    ALU.mult, ALU.add)
                lg2 = wk.tile([EPP, 64], F32, tag="lg2")
                nc.vector.scalar_tensor_tensor(
                    lg2[:], lg[:], 0.2, lg[:], ALU.mult, ALU.max)
                p_e = wk.tile([EPP, 64], vdt, tag="p_e")
                nc.scalar.activation(p_e[:], lg2[:], ACTF.Exp)

                p_self = None
                if self_loops:
                    sae = wk.tile([128, 64], F32, tag="sae")
                    nc.gpsimd.tensor_tensor(
                        sae[:].rearrange("p (a h) -> p a h", a=8),
                        c_webn[:, (li - 1) * 64:li * 64]
                        .rearrange("p (a h) -> p a h", a=8),
                        ea_loop[:][:, :, None].broadcast_to([128, 8, H]),
                        ALU.mult)
                    s1 = wk.tile([128, 64], F32, tag="s1")
                    nc.gpsimd.tensor_tensor(
                        s1[:].rearrange("p (a h) -> p a h", a=8),
                        hAv[:, :, 256:264],
                        hAv[:, :, 264:272], ALU.add)
                    s2 = wk.tile([128, 64], F32, tag="s2")
                    nc.gpsimd.tensor_tensor(
                        s2[:], s1[:], sae[:], ALU.add)
                    s3 = wk.tile([128, 64], F32, tag="s3")
                    nc.vector.scalar_tensor_tensor(
                        s3[:], s2[:], 0.2, s2[:], ALU.mult, ALU.max)
                    p_self = wk.tile([128, 64], F32, tag="p_self")
                    nc.scalar.activation(p_self[:], s3[:], ACTF.Exp)

                # ---- denominators ----
                for j in range(8):
                    nc.tensor.matmul(sp[:, DEN_ + j * 8:DEN_ + j * 8 + 8],
                                     dtt_l[j][:], p_e[:, j * 8:j * 8 + 8],
                                     start=True, stop=True)
                dtot = wk.tile([128, 64], F32, tag="dtot")
                if self_loops:
                    nc.vector.tensor_tensor(dtot[:], sp[:, DEN_:DEN_ + 64],
                                            p_self[:], ALU.add)
                else:
                    nc.vector.tensor_scalar(dtot[:], sp[:, DEN_:DEN_ + 64],
                                            1e-16, None, ALU.add)
                rden = wk.tile([128, 64], F32, tag="rden")
                nc.vector.reciprocal(rden[:], dtot[:])
                rden_v = wk.tile([128, 64], vdt, tag="rden_v")
                nc.scalar.copy(rden_v[:], rden[:])
                for j in range(8):
                    nc.tensor.matmul(sp[0:112, RD_ + j * 8:RD_ + j * 8 + 8],
                                     dblk_l[j], rden_v[:, j * 8:j * 8 + 8],
                                     start=True, stop=True)
                pn = wk.tile([EPP, 64], vdt, tag="pn")
                nc.vector.tensor_tensor(pn[:], sp[0:112, RD_:RD_ + 64], p_e[:],
                                        ALU.mult)
                if self_loops:
                    psn = wk.tile([128, 64], vdt, tag="psn")
                    nc.gpsimd.tensor_tensor(psn[:], p_self[:], rden[:], ALU.mult)

                # ---- phase B per duet: gather h, messages, scatter, relu ----
                hT_new = []
                for d in range(4):
                    g2 = ps.tile([EPP, 512], F32, tag="g2", bufs=2)
                    for jj in range(2):
                        j = 2 * d + jj
                        nc.tensor.matmul(g2[:, jj * 256:jj * 256 + 256],
                                         sblk_l[j], hAv[:, j, 0:256],
                                         start=True, stop=True)
                    msg2 = wk.tile([EPP, 512], vdt, tag="msg2")
                    nc.vector.tensor_tensor(
                        msg2[:].rearrange("p (a h c) -> p a h c", a=2, h=H),
                        g2[:, :].rearrange("p (a h c) -> p a h c", a=2, h=H),
                        pn[:, d * 16:(d + 1) * 16]
                        .rearrange("p (a h) -> p a h", a=2)[:, :, :, None]
                        .broadcast_to([EPP, 2, H, HID]), ALU.mult)
                    if self_loops:
                        msgs2 = wk.tile([128, 512], vdt, tag="msgs2")
                        for jj in range(2):
                            j = 2 * d + jj
                            nc.gpsimd.tensor_tensor(
                                msgs2[:, jj * 256:(jj + 1) * 256]
                                .rearrange("p (h c) -> p h c", h=H),
                                hAv[:, j, 0:256]
                                .rearrange("p (h c) -> p h c", h=H),
                                psn[:, j * 8:(j + 1) * 8][:, :, None]
                                .broadcast_to([128, H, HID]), ALU.mult)
                    o2 = ps.tile([128, 512], F32, tag="out2", bufs=1)
                    for jj in range(2):
                        j = 2 * d + jj
                        for c in range(2):
                            cs = slice(jj * 256 + c * 128, jj * 256 + c * 128 + 128)
                            nc.tensor.matmul(
                                o2[:, cs], msg2[:, jj * 256 + c * 128:
                                                jj * 256 + (c + 1) * 128],
                                dtt_l[j][:], start=True, stop=not self_loops)
                            if self_loops:
                                nc.tensor.matmul(
                                    o2[:, cs], msgs2[:, jj * 256 + c * 128:
                                                     jj * 256 + (c + 1) * 128],
                                    c_id[:, :], start=False, stop=True)
                    hT2 = wk.tile([128, 512], vdt, tag=f"hT{li}_{d}")
                    nc.scalar.activation(hT2[:], o2[:], ACTF.Relu)
                    hT_new.append(hT2)
                    for jj in range(2):
                        j = 2 * d + jj
                        prevT[j] = [hT2[:, jj * 256:jj * 256 + 128],
                                    hT2[:, jj * 256 + 128:jj * 256 + 256]]
                hT_l3 = hT_new

            # ---- pooling + MLP (octet-batched) ----
            zmlp = ps.tile([128, 144], F32, tag="g2", bufs=2)
            gev_o = wk.tile([128, 32], F32, tag="gev_o")   # (c, pair, g)
            gvv = gev_o[:, :].rearrange("p (c pr g) -> p pr c g", c=2, g=2)
            for d in range(4):
                nc.vector.tensor_reduce(
                    gvv[:, 2 * d:2 * d + 2, :, :],
                    hT_l3[d][:, :].rearrange(
                        "p (a b g n) -> p a b g n", a=2, b=2, g=2),
                    mybir.AxisListType.X, ALU.add)
            gev_v = wk.tile([128, 32], vdt, tag="gev_v")
            nc.scalar.copy(gev_v[:], gev_o[:])
            for d in range(4):
                for c in range(2):
                    agent = hT_l3[d][:, :].rearrange(
                        "p (a b g n) -> p a b g n", a=2, b=2, g=2)[:, :, c, :, 0:8]
                    nc.tensor.matmul(zmlp[:, Z_ + d * 32:Z_ + (d + 1) * 32],
                                     c_fc1a[:, bass.ts(c, 128)], agent,
                                     start=(c == 0), stop=(c == 1))
            for c in range(2):
                nc.tensor.matmul(zmlp[:, ZG_:ZG_ + 16],
                                 c_fc1g[:, bass.ts(c, 128)],
                                 gev_v[:, c * 16:(c + 1) * 16],
                                 start=(c == 0), stop=(c == 1))
            zgb = wk.tile([128, 16], F32, tag="zgb")
            nc.vector.scalar_tensor_tensor(
                zgb[:], zmlp[:, ZG_:ZG_ + 16], 1.0,
                c_fc1b[:, 0:1].broadcast_to([128, 16]), ALU.mult, ALU.add)
            zt = wk.tile([128, 128], F32, tag="zt")
            nc.vector.scalar_tensor_tensor(
                zt[:].rearrange("p (a b) -> p a b", a=16),
                zmlp[:, Z_:Z_ + 128].rearrange("p (a b) -> p a b", a=16), 1.0,
                zgb[:][:, :, None].broadcast_to([128, 16, 8]),
                ALU.mult, ALU.add)
            zbat = wk.tile([128, 128], vdt, tag="zbat")
            nc.scalar.activation(zbat[:], zt[:], ACTF.Relu)
            nc.tensor.matmul(zmlp[0:OUT, 0:128], c_fc2w[:, :], zbat[:],
                             start=True, stop=True)
            nc.vector.tensor_scalar(out_acc[:, oct_i * 128:(oct_i + 1) * 128],
                                    zmlp[0:OUT, 0:128], c_fc2b[:, 0:1], None,
                                    ALU.add)

        nc.sync.dma_start(out_d[:, :], out_acc[:])

    nc.compile()
    return nc


# ---------------- host-side packing ----------------

def _np_vdt(vdt):
    import ml_dtypes
    return {mybir.dt.bfloat16: ml_dtypes.bfloat16,
            mybir.dt.float32: np.float32}[vdt]


def host_prep(inputs, npairs=GPC // 2, vdt=mybir.dt.bfloat16):
    nv = _np_vdt(vdt)
    x = np.asarray(inputs["x"], np.float32)
    ei = np.asarray(inputs["edge_index"])
    eattr = np.asarray(inputs["edge_attr"], np.float32)
    for l in (1, 2, 3):
        assert not np.any(np.asarray(inputs[f"b{l}"])), "GAT bias must be 0"

    def pack_w(l):
        W = np.asarray(inputs[f"W{l}"], np.float32)
        a_s = np.asarray(inputs[f"as{l}"], np.float32)
        a_d = np.asarray(inputs[f"ad{l}"], np.float32)
        Ps = np.einsum("fkc,kc->fk", W.reshape(W.shape[0], H, HID), a_s)
        Pd = np.einsum("fkc,kc->fk", W.reshape(W.shape[0], H, HID), a_d)
        return np.concatenate([W, Ps, Pd], axis=1).astype(nv)

    def w_e(l):
        We = np.asarray(inputs[f"We{l}"], np.float32).reshape(H, HID)
        a_e = np.asarray(inputs[f"ae{l}"], np.float32)
        return (We * a_e).sum(-1)

    waug = {l: pack_w(l) for l in (1, 2, 3)}
    for l in (2, 3):
        waug[l] = np.concatenate([waug[l][:128], waug[l][128:]], axis=1)
    wev = np.concatenate([np.tile(w_e(l), 8) for l in (1, 2, 3)])   # [192]
    webe = np.broadcast_to(wev, (EPP, 192)).astype(np.float32).copy()
    webn = np.broadcast_to(wev, (128, 192)).astype(np.float32).copy()
    fc1_w = np.asarray(inputs["fc1_w"], np.float32)
    fc1a = np.concatenate([fc1_w[:128], fc1_w[128:HC]], axis=1).astype(nv)
    fc1g = np.concatenate([fc1_w[HC:HC + 128] / P,
                           fc1_w[HC + 128:] / P], axis=1).astype(nv)
    fc1b = np.asarray(inputs["fc1_b"], np.float32).reshape(128, 1)
    fc2w = np.asarray(inputs["fc2_w"], np.float32).astype(nv)
    fc2b = np.asarray(inputs["fc2_b"], np.float32).reshape(OUT, 1)
    identm = np.eye(128, dtype=np.float32).astype(nv)
    iota = np.broadcast_to(np.arange(128, dtype=np.float32),
                           (EPP, 128)).astype(nv).copy()
    iotac = np.arange(128, dtype=np.float32).reshape(128, 1)

    maps = []
    npc = GPC * P
    epc = GPC * OBS
    for m in range(NCORES):
        nsl = slice(m * npc, (m + 1) * npc)
        esl = slice(m * epc, (m + 1) * epc)
        xt = np.ascontiguousarray(x[nsl].T).astype(nv)
        src = np.asarray(ei[0][esl], np.int64) - m * npc
        dst = np.asarray(ei[1][esl], np.int64) - m * npc
        pairs = np.arange(GPC // 2).repeat(EPP)
        src_l = (src.reshape(-1) - pairs * 128).astype(np.float32)
        dst_l = (dst.reshape(-1) - pairs * 128).astype(np.float32)
        esrcb = np.ascontiguousarray(src_l.reshape(-1, EPP)).astype(nv)
        edstb = np.ascontiguousarray(dst_l.reshape(-1, EPP)).astype(nv)
        edst = np.ascontiguousarray(dst_l.reshape(-1, EPP).T)
        eat = np.ascontiguousarray(eattr[esl].reshape(-1, EPP).T).astype(np.float32)
        eap_arr = np.empty((EPP, 2 * npairs), np.float32)
        eap_arr[:, 0::2] = eat[:, :npairs]
        eap_arr[:, 1::2] = 1.0
        maps.append({
            "xt": xt[:, :npairs * 128],
            "esrcb": esrcb[:npairs], "edstb": edstb[:npairs],
            "edst": edst[:, :npairs],
            "eattr": eat[:, :npairs], "eap": eap_arr.astype(nv),
            "waug1": waug[1], "waug2": waug[2], "waug3": waug[3],
            "webe": webe, "webn": webn,
            "fc1a": fc1a, "fc1g": fc1g, "fc1b": fc1b,
            "fc2w": fc2w, "fc2b": fc2b,
            "ident": identm, "iota": iota, "iotac": iotac,
        })
    return maps


def unpack_out(res_list, npairs=GPC // 2):
    outs = []
    for m in range(NCORES):
        o = res_list[m]["out"]
        o = o.reshape(OUT, npairs, 2, A).transpose(1, 2, 3, 0)
        outs.append(o.reshape(npairs * 2, A, OUT))
    return np.concatenate(outs, axis=0).astype(np.float32)


# ---------------- entry point ----------------

LAST_EXEC_NS = None
LAST_TRACE = None
_NC_CACHE = {}


def _install_trace_hook():
    """Best-effort: register the axon NTFF profile hook so trace=True works.

    The agent image's antenv lacks axon_hooks; fabricate it and wire the
    ctypes hook from trn_agent_boot. Silently a no-op anywhere else.
    """
    try:
        import sys
        import types
        if 'antenv.axon_hooks' not in sys.modules:
            import antenv
            mod = types.ModuleType('antenv.axon_hooks')
            _h = [None]
            mod.set_axon_ntff_profile_hook = lambda h: _h.__setitem__(0, h)
            mod.get_axon_ntff_profile_hook = lambda: _h[0]
            sys.modules['antenv.axon_hooks'] = mod
            antenv.axon_hooks = mod
        import antenv.axon_hooks as ah
        if ah.get_axon_ntff_profile_hook() is None:
            if '/root/.axon_site' not in sys.path:
                sys.path.insert(0, '/root/.axon_site')
            from trn_agent_boot.trn_boot import _ntff_profile_via_ctypes
            hook = _ntff_profile_via_ctypes('/opt/axon/libaxon_pjrt.so')
            if hook is not None:
                ah.set_axon_ntff_profile_hook(hook)
    except Exception:
        pass


def kernel(**inputs) -> np.ndarray:
    """Full-input GAT forward on 8 NeuronCores; returns [4096, 8, 2] f32."""
    global LAST_EXEC_NS, LAST_TRACE
    import os
    vdt = mybir.dt.bfloat16
    npairs = GPC // 2
    key = (npairs, vdt)
    if key not in _NC_CACHE:
        _NC_CACHE[key] = build(npairs, vdt=vdt, num_devices=NCORES)
    nc = _NC_CACHE[key]
    maps = host_prep(inputs, npairs=npairs, vdt=vdt)
    trace = os.environ.get("BASS_GAT_TRACE") == "1"
    if trace:
        _install_trace_hook()
    res = None
    for attempt in range(3):
        try:
            res = run_bass_kernel_spmd(
                nc, maps, core_ids=list(range(NCORES)),
                trace=trace and attempt == 0,
                trace_cores=[0] if trace and attempt == 0 else None)
            break
        except Exception:
            if attempt == 2:
                raise
            import time
            time.sleep(10)
    LAST_EXEC_NS = res.exec_time_ns
    LAST_TRACE = res.instructions_and_trace
    return unpack_out([r for r in res.results], npairs=npairs)
